# revision 1
# baseline (speedup 1.0000x reference)
"""MetaSage GNN kernel for 8 Trainium2 NeuronCores (Bass/Tile, SPMD).

Strategy (graph/edge parallel, dst-node sharded):
- Destination nodes sharded contiguously across 8 cores (products 12500/core,
  customers 6250/core). Edges bucketed by dst shard on host, sorted by dst,
  grouped into dst-tiles of 128 nodes, each tile's edge list padded to M
  chunks of 128 edges (M = global max, uniform -> single SPMD program).
- Per chunk: indirect-DMA gather of 128 source rows (512B each), one-hot
  [edge, dst] built on DVE (is_equal vs iota), segment-sum via PE matmul
  accumulating into PSUM [128 dst, 128 feat + 1 count col].
- mean = sum/max(cnt,1); SAGE linear in transposed orientation
  h = relu(Wl @ meanT + b + Wr @ xT) on PE; layer-1 aggregation shared
  between item and user encoders.
- AllGather (p, px) feeds layer-2 gathers; decoder linears folded into the
  node-level linears on host (z_cust/z_prod never materialized):
    ZC' = cx2 @ (W1L us_Wlin).T + (de_b1 + W1L us_blin + W1R it_blin)
    ZP' = p2 @ (W1R it_Wlin).T
    out[e] = w2 . relu(ZC'[row] + ZP'[col]) + de_b2
- Decoder: gather ZC'/ZP' rows, add+relu+mul+grouped-reduce on DVE.
"""
import numpy as np
from contextlib import ExitStack

from concourse import bass, bacc, mybir
from concourse import bass_utils
import concourse.tile as tile
from concourse.masks import make_identity

P = 128
NCORES = 8
N_PROD, N_CUST = 100000, 50000
HID, OUT = 128, 64
E_LB = 400000
PS = N_PROD // NCORES          # 12500 product dsts per core
CS = N_CUST // NCORES          # 6250 customer dsts per core
PT = (PS + P - 1) // P         # 98 tiles
CT = (CS + P - 1) // P         # 49 tiles
PSP = PT * P                   # 12544 padded product shard
CSP = CT * P                   # 6272 padded customer shard
PFULL = NCORES * PSP           # 100352
CFULL = NCORES * CSP           # 50176
EC = E_LB // NCORES            # 50000 label edges per core
DEC = (EC + P - 1) // P        # 391 chunks
DECN = DEC * P                 # 50048
GD = 4                         # decoder chunk grouping
F32 = mybir.dt.float32
I32 = mybir.dt.int32

_cache = {}


def _bucket_edges(src, dst, S, T):
    """Bucket edges by dst shard, sort by local dst. -> per-core (srcs, ldst)."""
    src = np.asarray(src).astype(np.int64)
    dst = np.asarray(dst).astype(np.int64)
    out = []
    core = dst // S
    for c in range(NCORES):
        m = core == c
        s_c, ld = src[m], dst[m] - c * S
        o = np.argsort(ld, kind="stable")
        out.append((s_c[o], ld[o]))
    return out


def _edge_tiles(buckets, T):
    """-> M (global chunks/tile), per-core (idx[128,T*M] int32 raw-src, doff)."""
    M = 1
    infos = []
    for s_c, ld in buckets:
        tid = ld >> 7
        cnt = np.bincount(tid, minlength=T)
        M = max(M, int((cnt.max() + P - 1) // P))
        starts = np.concatenate([[0], np.cumsum(cnt)])
        k = np.arange(len(ld)) - starts[tid]
        infos.append((s_c, ld, tid, k))
    idxs, doffs = [], []
    for s_c, ld, tid, k in infos:
        col = tid * M + (k >> 7)
        row = k & 127
        idx = np.zeros((P, T * M), np.int32)
        doff = np.full((P, T * M), -1.0, np.float32)
        idx[row, col] = s_c
        doff[row, col] = (ld - (tid << 7)).astype(np.float32)
        idxs.append(idx)
        doffs.append(doff)
    return M, idxs, doffs


def _remap_prod(g):
    return ((g // PS) * PSP + g % PS).astype(np.int32)


def _remap_cust(g):
    return ((g // CS) * CSP + g % CS).astype(np.int32)


def build_program(M1, M2):
    key = (M1, M2)
    if key in _cache:
        return _cache[key]
    nc = bacc.Bacc("TRN2", target_bir_lowering=False, debug=False,
                   num_devices=NCORES)
    Mmax = max(M1, M2)

    ein = lambda n, s, d=F32: nc.dram_tensor(n, s, d, kind="ExternalInput")
    x_shard = ein("x_shard", [PSP, HID])
    xprodT = ein("xprodT", [P, PSP])
    xcustT = ein("xcustT", [P, CSP])
    idx_pp = ein("idx_pp", [P, PT * M1], I32)
    doff_pp = ein("doff_pp", [P, PT * M1])
    idx_pc = ein("idx_pc", [P, CT * M2], I32)
    doff_pc = ein("doff_pc", [P, CT * M2])
    dec_row = ein("dec_row", [P, DEC], I32)
    dec_col = ein("dec_col", [P, DEC], I32)
    wnames = ["itW1lT", "itW1rT", "usW1lT", "usW1rT", "itW2lT", "itW2rT",
              "usW2lT", "usW2rT", "usW3lT", "usW3rT"]
    wts = {n: ein(n, [HID, HID]) for n in wnames}
    WpT = ein("WpT", [HID, OUT])
    WcT = ein("WcT", [HID, OUT])
    bias = {n: ein(n, [HID, 1]) for n in ["itb1", "usb1", "itb2", "usb2", "usb3"]}
    bc = ein("bc", [OUT, 1])
    w2rep = ein("w2rep", [P, OUT])
    b2rep = ein("b2rep", [P, 1])
    out = nc.dram_tensor("out", [DECN, 1], F32, kind="ExternalOutput")

    with tile.TileContext(nc) as tc, ExitStack() as ctx:
        dram = ctx.enter_context(tc.tile_pool(name="dram", bufs=1, space="DRAM"))
        cst = ctx.enter_context(tc.tile_pool(name="cst", bufs=1))
        res = ctx.enter_context(tc.tile_pool(name="res", bufs=1))
        sb = ctx.enter_context(tc.tile_pool(name="sb", bufs=2))
        msgp = ctx.enter_context(tc.tile_pool(name="msgp", bufs=2))
        ps = ctx.enter_context(tc.tile_pool(name="ps", bufs=2, space="PSUM"))

        # DRAM intermediates (collective buffers)
        p_shard = dram.tile([PSP, HID], F32)
        px_shard = dram.tile([PSP, HID], F32)
        zp_shard = dram.tile([PSP, OUT], F32)
        zc_shard = dram.tile([CSP, OUT], F32)
        p_full = dram.tile([PFULL, HID], F32, addr_space="Shared")
        px_full = dram.tile([PFULL, HID], F32, addr_space="Shared")
        zp_full = dram.tile([PFULL, OUT], F32, addr_space="Shared")
        zc_full = dram.tile([CFULL, OUT], F32, addr_space="Shared")
        x_sh_int = dram.tile([PSP, HID], F32)
        x_full = dram.tile([PFULL, HID], F32, addr_space="Shared")

        # constants
        ident = cst.tile([P, P], F32)
        make_identity(nc, ident[:])
        iota_i = cst.tile([P, Mmax * P], I32)
        nc.gpsimd.iota(iota_i[:].rearrange("p (m f) -> p m f", f=P),
                       pattern=[[0, Mmax], [1, P]], base=0, channel_multiplier=0)
        iota_f = cst.tile([P, Mmax * P], F32)
        nc.vector.tensor_copy(out=iota_f[:], in_=iota_i[:])

        def load_const(t, shape=None):
            dst = cst.tile(shape or t.shape, t.dtype, tag=t.name)
            nc.sync.dma_start(out=dst[:], in_=t[:, :])
            return dst

        w_t = {n: load_const(w) for n, w in wts.items()}
        WpT_t, WcT_t = load_const(WpT), load_const(WcT)
        b_t = {n: load_const(b) for n, b in bias.items()}
        bc_t, w2_t, b2_t = load_const(bc), load_const(w2rep), load_const(b2rep)
        idxpp_t = load_const(idx_pp)
        doffpp_t = load_const(doff_pp)
        idxpc_t = load_const(idx_pc)
        doffpc_t = load_const(doff_pc)
        decr_t, decc_t = load_const(dec_row), load_const(dec_col)

        # residents
        p_res = res.tile([P, PSP], F32)    # p.T (item layer1 out)
        cx_res = res.tile([P, CSP], F32)   # cx.T (user cust layer1 out)

        def sage_pass(ntiles, M, idx_t, doff_t, table_ap, self_rhs, branches):
            """branches: list of (WlT_ap, WrT_ap, bias_ap, sink(t, pl_psum))"""
            for t in range(ntiles):
                msg = msgp.tile([P, M * 129], F32, tag="msg")
                msg3 = msg[:].rearrange("p (m f) -> p m f", f=129)
                nc.vector.memset(msg3[:, :, 128:129], 1.0)
                for m in range(M):
                    k = t * M + m
                    nc.gpsimd.indirect_dma_start(
                        out=msg3[:, m, 0:128], out_offset=None, in_=table_ap,
                        in_offset=bass.IndirectOffsetOnAxis(
                            ap=idx_t[:, k:k + 1], axis=0))
                oh = msgp.tile([P, M * P], F32, tag="oh")
                nc.vector.tensor_tensor(
                    out=oh[:].rearrange("p (m f) -> p m f", f=P),
                    in0=doff_t[:, t * M:(t + 1) * M, None].to_broadcast([P, M, P]),
                    in1=iota_f[:, 0:M * P].rearrange("p (m f) -> p m f", f=P),
                    op=mybir.AluOpType.is_equal)
                pagg = ps.tile([P, 129], F32, tag="pagg", space="PSUM")
                for m in range(M):
                    nc.tensor.matmul(out=pagg[:], lhsT=oh[:, m * P:(m + 1) * P],
                                     rhs=msg3[:, m, :], start=(m == 0),
                                     stop=(m == M - 1))
                inv = sb.tile([P, 1], F32, tag="inv")
                nc.vector.tensor_scalar_max(out=inv[:], in0=pagg[:, 128:129],
                                            scalar1=1.0)
                nc.vector.reciprocal(out=inv[:], in_=inv[:])
                mean = sb.tile([P, P], F32, tag="mean")
                nc.vector.tensor_scalar_mul(out=mean[:], in0=pagg[:, 0:128],
                                            scalar1=inv[:, 0:1])
                mT_ps = ps.tile([P, P], F32, tag="pmT", space="PSUM")
                nc.tensor.transpose(out=mT_ps[:], in_=mean[:], identity=ident[:])
                mT = sb.tile([P, P], F32, tag="mT")
                nc.vector.tensor_copy(out=mT[:], in_=mT_ps[:])
                xT = self_rhs(t)
                for WlT_ap, WrT_ap, bias_ap, sink in branches:
                    pl = ps.tile([P, P], F32, tag="plin", space="PSUM")
                    nc.tensor.matmul(out=pl[:], lhsT=WlT_ap, rhs=mT[:],
                                     start=True, stop=False)
                    nc.tensor.matmul(out=pl[:], lhsT=WrT_ap, rhs=xT,
                                     start=False, stop=True)
                    sink(t, pl, bias_ap)

        def sink_store(resid, dram_tile):
            """relu into resident [o,d] slice; optionally transpose out to DRAM"""
            def f(t, pl, bias_ap):
                if resid is not None:
                    h = resid[:, t * P:(t + 1) * P]
                    nc.scalar.activation(out=h, in_=pl[:],
                                         func=mybir.ActivationFunctionType.Relu,
                                         bias=bias_ap)
                else:
                    ht = sb.tile([P, P], F32, tag="h")
                    h = ht[:]
                    nc.scalar.activation(out=h, in_=pl[:],
                                         func=mybir.ActivationFunctionType.Relu,
                                         bias=bias_ap)
                if dram_tile is not None:
                    tp = ps.tile([P, P], F32, tag="ptr", space="PSUM")
                    nc.tensor.transpose(out=tp[:], in_=h, identity=ident[:])
                    hT = sb.tile([P, P], F32, tag="hT")
                    nc.vector.tensor_copy(out=hT[:], in_=tp[:])
                    nc.sync.dma_start(out=dram_tile[t * P:(t + 1) * P, :],
                                      in_=hT[:])
            return f

        def sink_z(WzT_ap, bz_ap, z_dram):
            """h2 = relu(pl); z = WzT.T @ h2 (+bz); transpose; DMA [d, OUT]"""
            def f(t, pl, bias_ap):
                ht = sb.tile([P, P], F32, tag="h")
                nc.scalar.activation(out=ht[:], in_=pl[:],
                                     func=mybir.ActivationFunctionType.Relu,
                                     bias=bias_ap)
                pz = ps.tile([OUT, P], F32, tag="plin", space="PSUM")
                nc.tensor.matmul(out=pz[:], lhsT=WzT_ap, rhs=ht[:],
                                 start=True, stop=True)
                zsb = sb.tile([OUT, P], F32, tag="zsb")
                if bz_ap is not None:
                    nc.vector.tensor_scalar_add(out=zsb[:], in0=pz[:],
                                                scalar1=bz_ap)
                else:
                    nc.vector.tensor_copy(out=zsb[:], in_=pz[:])
                tp = ps.tile([P, OUT], F32, tag="ptr", space="PSUM")
                nc.tensor.transpose(out=tp[:], in_=zsb[:],
                                    identity=ident[0:OUT, 0:OUT])
                zT = sb.tile([P, OUT], F32, tag="hT")
                nc.vector.tensor_copy(out=zT[:], in_=tp[:])
                nc.sync.dma_start(out=z_dram[t * P:(t + 1) * P, :], in_=zT[:])
            return f

        def stream_selfT(src_dram):
            def f(t):
                xt = sb.tile([P, P], F32, tag="xT")
                nc.sync.dma_start(out=xt[:], in_=src_dram[:, t * P:(t + 1) * P])
                return xt[:]
            return f

        # ---- AllGather x_product shards -> x_full
        rg = [list(range(NCORES))]
        for blk in range(PT):
            xb = sb.tile([P, HID], F32, tag='xbounce')
            nc.sync.dma_start(out=xb[:], in_=x_shard[blk * P:(blk + 1) * P, :])
            nc.sync.dma_start(out=x_sh_int[blk * P:(blk + 1) * P, :], in_=xb[:])
        nc.gpsimd.collective_compute("AllGather", mybir.AluOpType.bypass,
                                     replica_groups=rg, ins=[x_sh_int.opt()],
                                     outs=[x_full.opt()])

        # ---- pass A1: pp edges -> mean1 -> p (item) & px (user), shared agg
        sage_pass(PT, M1, idxpp_t[:], doffpp_t[:], x_full[:],
                  stream_selfT(xprodT),
                  [(w_t["itW1lT"][:], w_t["itW1rT"][:], b_t["itb1"][:, 0:1],
                    sink_store(p_res[:], p_shard)),
                   (w_t["usW1lT"][:], w_t["usW1rT"][:], b_t["usb1"][:, 0:1],
                    sink_store(None, px_shard))])

        # ---- pass B1: pc edges (x_prod -> cust) -> cx resident
        sage_pass(CT, M2, idxpc_t[:], doffpc_t[:], x_full[:],
                  stream_selfT(xcustT),
                  [(w_t["usW2lT"][:], w_t["usW2rT"][:], b_t["usb2"][:, 0:1],
                    sink_store(cx_res[:], None))])

        # ---- AllGather p, px
        nc.gpsimd.collective_compute("AllGather", mybir.AluOpType.bypass,
                                     replica_groups=rg, ins=[p_shard.opt()],
                                     outs=[p_full.opt()])
        nc.gpsimd.collective_compute("AllGather", mybir.AluOpType.bypass,
                                     replica_groups=rg, ins=[px_shard.opt()],
                                     outs=[px_full.opt()])

        # ---- pass A2: pp edges over p -> p2 -> ZP'
        sage_pass(PT, M1, idxpp_t[:], doffpp_t[:], p_full[:],
                  lambda t: p_res[:, t * P:(t + 1) * P],
                  [(w_t["itW2lT"][:], w_t["itW2rT"][:], b_t["itb2"][:, 0:1],
                    sink_z(WpT_t[:], None, zp_shard))])

        # ---- pass B2: pc edges over px -> cx2 -> ZC'
        sage_pass(CT, M2, idxpc_t[:], doffpc_t[:], px_full[:],
                  lambda t: cx_res[:, t * P:(t + 1) * P],
                  [(w_t["usW3lT"][:], w_t["usW3rT"][:], b_t["usb3"][:, 0:1],
                    sink_z(WcT_t[:], bc_t[:, 0:1], zc_shard))])

        # ---- AllGather ZP', ZC'
        nc.gpsimd.collective_compute("AllGather", mybir.AluOpType.bypass,
                                     replica_groups=rg, ins=[zp_shard.opt()],
                                     outs=[zp_full.opt()])
        nc.gpsimd.collective_compute("AllGather", mybir.AluOpType.bypass,
                                     replica_groups=rg, ins=[zc_shard.opt()],
                                     outs=[zc_full.opt()])

        # ---- decoder
        acc = res.tile([P, DEC], F32)
        ngroups = (DEC + GD - 1) // GD
        for g in range(ngroups):
            w = min(GD, DEC - g * GD)
            zcq = sb.tile([P, GD * OUT], F32, tag="zcq")
            zpq = sb.tile([P, GD * OUT], F32, tag="zpq")
            for j in range(w):
                c = g * GD + j
                nc.gpsimd.indirect_dma_start(
                    out=zcq[:, j * OUT:(j + 1) * OUT], out_offset=None,
                    in_=zc_full[:],
                    in_offset=bass.IndirectOffsetOnAxis(
                        ap=decr_t[:, c:c + 1], axis=0))
                nc.gpsimd.indirect_dma_start(
                    out=zpq[:, j * OUT:(j + 1) * OUT], out_offset=None,
                    in_=zp_full[:],
                    in_offset=bass.IndirectOffsetOnAxis(
                        ap=decc_t[:, c:c + 1], axis=0))
            sq = sb.tile([P, GD * OUT], F32, tag="sq")
            nc.vector.tensor_tensor(out=sq[:, 0:w * OUT], in0=zcq[:, 0:w * OUT],
                                    in1=zpq[:, 0:w * OUT],
                                    op=mybir.AluOpType.add)
            rq = sb.tile([P, GD * OUT], F32, tag="rq")
            nc.scalar.activation(out=rq[:, 0:w * OUT], in_=sq[:, 0:w * OUT],
                                 func=mybir.ActivationFunctionType.Relu)
            mq = sb.tile([P, GD * OUT], F32, tag="mq")
            nc.vector.tensor_tensor(
                out=mq[:].rearrange("p (j f) -> p j f", f=OUT)[:, 0:w, :],
                in0=rq[:].rearrange("p (j f) -> p j f", f=OUT)[:, 0:w, :],
                in1=w2_t[:, None, 0:OUT].to_broadcast([P, w, OUT]),
                op=mybir.AluOpType.mult)
            nc.vector.reduce_sum(
                out=acc[:, g * GD:g * GD + w],
                in_=mq[:].rearrange("p (j f) -> p j f", f=OUT)[:, 0:w, :],
                axis=mybir.AxisListType.X)
        acc_b = res.tile([P, DEC], F32)
        nc.vector.tensor_scalar_add(out=acc_b[:], in0=acc[:],
                                    scalar1=b2_t[:, 0:1])
        outv = out[:, :].rearrange("(c p) o -> c (p o)", p=P)
        for b in range((DEC + P - 1) // P):
            w = min(P, DEC - b * P)
            tp = ps.tile([P, P], F32, tag="ptr", space="PSUM")
            nc.tensor.transpose(out=tp[0:w, :], in_=acc_b[:, b * P:b * P + w],
                                identity=ident[:])
            ts = sb.tile([P, P], F32, tag="hT")
            nc.vector.tensor_copy(out=ts[0:w, :], in_=tp[0:w, :])
            nc.sync.dma_start(out=outv[b * P:b * P + w, :], in_=ts[0:w, :])

    nc.compile()
    _cache[(M1, M2)] = nc
    return nc


def kernel(**inputs):
    x_product = np.ascontiguousarray(np.asarray(inputs["x_product"], np.float32))
    x_customer = np.ascontiguousarray(np.asarray(inputs["x_customer"], np.float32))
    ei_pp = np.asarray(inputs["ei_pp"])
    ei_pc = np.asarray(inputs["ei_pc"])
    eli = np.asarray(inputs["edge_label_index"])

    # host prep: edge bucketing (sharding) + weight folding
    bpp = _bucket_edges(ei_pp[0], ei_pp[1], PS, PT)
    bpc = _bucket_edges(ei_pc[0], ei_pc[1], CS, CT)
    M1, idx_pp, doff_pp = _edge_tiles(bpp, PT)
    M2, idx_pc, doff_pc = _edge_tiles(bpc, CT)
    idx_pp = [_remap_prod(a.astype(np.int64)) for a in idx_pp]
    idx_pc = [_remap_prod(a.astype(np.int64)) for a in idx_pc]

    row, col = eli[0].astype(np.int64), eli[1].astype(np.int64)
    dec_rows, dec_cols = [], []
    for c in range(NCORES):
        r = np.zeros(DECN, np.int64)
        q = np.zeros(DECN, np.int64)
        r[:EC] = row[c * EC:(c + 1) * EC]
        q[:EC] = col[c * EC:(c + 1) * EC]
        dec_rows.append(np.ascontiguousarray(
            _remap_cust(r).reshape(DEC, P).T))
        dec_cols.append(np.ascontiguousarray(
            _remap_prod(q).reshape(DEC, P).T))

    f32 = lambda a: np.ascontiguousarray(np.asarray(a, np.float32))
    W = {k: f32(inputs[k]) for k in
         ["it_W1l", "it_W1r", "it_W2l", "it_W2r", "it_Wlin",
          "us_W1l", "us_W1r", "us_W2l", "us_W2r", "us_W3l", "us_W3r",
          "us_Wlin", "de_W1", "de_W2"]}
    b = {k: f32(inputs[k]) for k in
         ["it_b1", "it_b2", "it_blin", "us_b1", "us_b2", "us_b3", "us_blin",
          "de_b1", "de_b2"]}
    W1L, W1R = W["de_W1"][:, :OUT], W["de_W1"][:, OUT:]
    shared = {
        "itW1lT": f32(W["it_W1l"].T), "itW1rT": f32(W["it_W1r"].T),
        "usW1lT": f32(W["us_W1l"].T), "usW1rT": f32(W["us_W1r"].T),
        "itW2lT": f32(W["it_W2l"].T), "itW2rT": f32(W["it_W2r"].T),
        "usW2lT": f32(W["us_W2l"].T), "usW2rT": f32(W["us_W2r"].T),
        "usW3lT": f32(W["us_W3l"].T), "usW3rT": f32(W["us_W3r"].T),
        "WpT": f32((W1R @ W["it_Wlin"]).T),
        "WcT": f32((W1L @ W["us_Wlin"]).T),
        "itb1": b["it_b1"].reshape(HID, 1), "usb1": b["us_b1"].reshape(HID, 1),
        "itb2": b["it_b2"].reshape(HID, 1), "usb2": b["us_b2"].reshape(HID, 1),
        "usb3": b["us_b3"].reshape(HID, 1),
        "bc": f32(b["de_b1"] + W1L @ b["us_blin"]
                  + W1R @ b["it_blin"]).reshape(OUT, 1),
        "w2rep": f32(np.tile(W["de_W2"].reshape(1, OUT), (P, 1))),
        "b2rep": np.full((P, 1), np.float32(b["de_b2"].reshape(-1)[0])),
    }
    xpad = np.zeros((PSP, HID), np.float32)
    cpad = np.zeros((CSP, HID), np.float32)
    in_maps = []
    for c in range(NCORES):
        xpad[:PS] = x_product[c * PS:(c + 1) * PS]
        cpad[:CS] = x_customer[c * CS:(c + 1) * CS]
        in_maps.append(dict(shared,
                            x_shard=xpad.copy(), xprodT=f32(xpad.T),
                            xcustT=f32(cpad.T),
                            idx_pp=idx_pp[c], doff_pp=doff_pp[c],
                            idx_pc=idx_pc[c], doff_pc=doff_pc[c],
                            dec_row=dec_rows[c], dec_col=dec_cols[c]))

    nc = build_program(M1, M2)
    res = bass_utils.run_bass_kernel_spmd(nc, in_maps,
                                          core_ids=list(range(NCORES)))
    kernel.last_in_maps = in_maps
    kernel.last_nc = nc
    return np.concatenate([res.results[c]["out"][:EC] for c in range(NCORES)],
                          axis=0).astype(np.float32)

